# revision 1
# baseline (speedup 1.0000x reference)
"""Cross-attention kernel for Trainium2, sharded over 8 NeuronCores.

Sharding: core c handles batch b = c // 4 and head-group g = c % 4
(4 of 16 heads, i.e. 256 of 1024 channels). Each core computes
  q_g = query[b] @ Wq[g].T ; k_g = key[b] @ Wk[g].T ; v_g = value[b] @ Wv[g].T
  x_g = softmax(q_g k_g^T * scale) v_g          (4 heads, independent)
  partial_g = x_g @ Wp[:, g].T                  (partial over head-group)
Host sums the 4 partials per batch and adds the bias terms
(bp exactly, and bv folded through: softmax rows sum to 1, so the value
bias contributes bv @ Wp.T to every token).

On-chip layout: channel-major ("transposed") activations so every matmul
contracts along SBUF partitions. Scores are computed transposed
(scoresT[m, q]) so the softmax denominator reduces along partitions via a
ones-augmented value matrix (extra column of 1.0 in v), and the PV matmul
chains directly off the exp output. All matmuls run in float32r (full PE
rate at free-dim >= 256).
"""

import numpy as np

import concourse.bass as bass
import concourse.mybir as mybir
import concourse.tile as tile
from concourse import bacc
from concourse.bass_utils import run_bass_kernel_spmd

B, N, DIM, H, DH = 2, 2048, 1024, 16, 64
NCORES = 8
HG = 4            # head-groups (cores per batch)
HPG = H // HG     # heads per group = 4
CS = DIM // HG    # channels per group = 256
P = 128
KT = DIM // P     # 8 contraction tiles for the projections
NT = N // P       # 16 token tiles
QW = 512          # q-chunk width (fp32 moving-operand max)
QC = N // QW      # 4 q-chunks

FP32 = mybir.dt.float32
FP32R = mybir.dt.float32r
AF = mybir.ActivationFunctionType


def _build(scale: float, add_qk_bias: bool, reps: int = 1,
           loop_reps: int | None = None):
    nc = bacc.Bacc("TRN2", target_bir_lowering=False, debug=False,
                   num_devices=NCORES)

    qT = nc.dram_tensor("qT", [DIM, N], FP32R, kind="ExternalInput").ap()
    kT = nc.dram_tensor("kT", [DIM, N], FP32R, kind="ExternalInput").ap()
    vT = nc.dram_tensor("vT", [DIM, N], FP32R, kind="ExternalInput").ap()
    wq = nc.dram_tensor("wq", [DIM, CS], FP32R, kind="ExternalInput").ap()
    wk = nc.dram_tensor("wk", [DIM, CS], FP32R, kind="ExternalInput").ap()
    wv = nc.dram_tensor("wv", [DIM, CS], FP32R, kind="ExternalInput").ap()
    wp = nc.dram_tensor("wp", [CS, DIM], FP32R, kind="ExternalInput").ap()
    bqk = nc.dram_tensor("bqk", [P, 2 * (CS // P)], FP32,
                         kind="ExternalInput").ap()
    out = nc.dram_tensor("out", [DIM, N], FP32, kind="ExternalOutput").ap()

    from contextlib import ExitStack
    with nc.allow_low_precision(reason="fp32r matmul rounding is intended"), \
         tile.TileContext(nc) as tc, ExitStack() as stack:
        wpool = stack.enter_context(tc.tile_pool(name="wpool", bufs=1))
        persist = stack.enter_context(tc.tile_pool(name="persist", bufs=1))
        const = stack.enter_context(tc.tile_pool(name="const", bufs=1))

        # Weights resident in SBUF.
        wq_sb = wpool.tile([P, KT * CS], FP32R, tag="wq")
        wk_sb = wpool.tile([P, KT * CS], FP32R, tag="wk")
        wv_sb = wpool.tile([P, KT * CS], FP32R, tag="wv")
        wp_sb = wpool.tile([P, (CS // P) * DIM], FP32R, tag="wp")
        for k in range(KT):
            nc.sync.dma_start(out=wq_sb[:, k * CS:(k + 1) * CS],
                              in_=wq[k * P:(k + 1) * P, :])
            nc.sync.dma_start(out=wk_sb[:, k * CS:(k + 1) * CS],
                              in_=wk[k * P:(k + 1) * P, :])
            nc.sync.dma_start(out=wv_sb[:, k * CS:(k + 1) * CS],
                              in_=wv[k * P:(k + 1) * P, :])
        for k2 in range(CS // P):
            nc.sync.dma_start(out=wp_sb[:, k2 * DIM:(k2 + 1) * DIM],
                              in_=wp[k2 * P:(k2 + 1) * P, :])
        bqk_sb = const.tile([P, 2 * (CS // P)], FP32, tag="bqk")
        if add_qk_bias:
            nc.sync.dma_start(out=bqk_sb[:], in_=bqk[:])
        ones = const.tile([1, DH], FP32R, tag="ones")
        nc.vector.memset(ones[:].bitcast(FP32), 1.0)

        # Projected activations, channel-major, fp32r.
        qsb = [persist.tile([P, N], FP32R, tag=f"qsb{t}", name=f"qsb{t}") for t in range(2)]
        ksb = [persist.tile([P, N], FP32R, tag=f"ksb{t}", name=f"ksb{t}") for t in range(2)]
        # v token-major with a ones column per head: [tok, 4*(64+1)]
        vsb = [persist.tile([P, HPG * (DH + 1)], FP32R, tag=f"vsb{t}", name=f"vsb{t}")
               for t in range(NT)]

        from contextlib import nullcontext
        loop_cm = (tc.For_i(0, loop_reps, 1) if loop_reps
                   else nullcontext())
        with loop_cm:
          for rep in range(reps):
            # ---- Phase Q / K: channel-major projections -------------------
            def qk_proj(src_dram, w_sb, dst, bias_col):
                with tc.tile_pool(name="stream", bufs=2) as stream, \
                     tc.tile_pool(name="pp", bufs=1, space="PSUM") as pp:
                    pA = pp.tile([P, N], FP32, tag="pA")
                    pB = pp.tile([P, N], FP32, tag="pB")
                    for k in range(KT):
                        ts_ = stream.tile([P, N], FP32R, tag="s")
                        nc.sync.dma_start(out=ts_[:],
                                          in_=src_dram[k * P:(k + 1) * P, :])
                        for nn in range(QC):
                            nc.tensor.matmul(
                                pA[:, nn * QW:(nn + 1) * QW],
                                w_sb[:, k * CS:k * CS + P],
                                ts_[:, nn * QW:(nn + 1) * QW],
                                start=(k == 0), stop=(k == KT - 1))
                            nc.tensor.matmul(
                                pB[:, nn * QW:(nn + 1) * QW],
                                w_sb[:, k * CS + P:(k + 1) * CS],
                                ts_[:, nn * QW:(nn + 1) * QW],
                                start=(k == 0), stop=(k == KT - 1))
                    for t, pt in enumerate((pA, pB)):
                        if add_qk_bias:
                            nc.vector.tensor_scalar(
                                dst[t][:], pt[:],
                                bqk_sb[:, bias_col + t:bias_col + t + 1], None,
                                mybir.AluOpType.add)
                        else:
                            nc.vector.tensor_copy(dst[t][:], pt[:])

            qk_proj(qT, wq_sb, qsb, 0)
            qk_proj(kT, wk_sb, ksb, CS // P)

            # ---- Phase V: token-major projection --------------------------
            # One PSUM bank per token-tile accumulator (start=True clears the
            # whole bank, so accumulation groups must not share one). 8 banks
            # per pass, two passes over a fully resident vT.
            with tc.tile_pool(name="streamv", bufs=1) as stream, \
                 tc.tile_pool(name="pv", bufs=8, space="PSUM") as pvp:
                vres = [stream.tile([P, N], FP32R, tag=f"vres{k}",
                                    name=f"vres{k}_{rep}") for k in range(KT)]
                for k in range(KT):
                    nc.sync.dma_start(out=vres[k][:], in_=vT[k * P:(k + 1) * P, :])
                for half in range(2):
                    pvt = [pvp.tile([P, CS], FP32, tag="pv",
                                    name=f"pv{half}_{t8}_{rep}") for t8 in range(8)]
                    for k in range(KT):
                        for t8 in range(8):
                            tt = half * 8 + t8
                            nc.tensor.matmul(
                                pvt[t8][:],
                                vres[k][:, tt * P:(tt + 1) * P],
                                wv_sb[:, k * CS:(k + 1) * CS],
                                start=(k == 0), stop=(k == KT - 1))
                    for t8 in range(8):
                        tt = half * 8 + t8
                        dst3 = vsb[tt][:].rearrange("p (h c) -> p h c", h=HPG)
                        nc.vector.tensor_copy(
                            dst3[:, :, 0:DH],
                            pvt[t8][:].rearrange("p (h c) -> p h c", h=HPG))
                        nc.vector.memset(dst3[:, :, DH:DH + 1].bitcast(FP32), 1.0)

            # ---- Phase C: attention + output projection, per q-chunk ------
            with tc.tile_pool(name="probs", bufs=3) as probs, \
                 tc.tile_pool(name="xq", bufs=2) as xqp, \
                 tc.tile_pool(name="small", bufs=2) as small, \
                 tc.tile_pool(name="ost", bufs=2) as ostp, \
                 tc.tile_pool(name="psc", bufs=2, space="PSUM") as psc, \
                 tc.tile_pool(name="pxt", bufs=1, space="PSUM") as pxt, \
                 tc.tile_pool(name="pbc", bufs=1, space="PSUM") as pbc, \
                 tc.tile_pool(name="po", bufs=1, space="PSUM") as pop:
                for qq in range(QC):
                    qs = slice(qq * QW, (qq + 1) * QW)
                    xq = [xqp.tile([P, QW], FP32R, tag=f"x{t}", name=f"xq{t}_{qq}_{rep}") for t in range(2)]
                    for hp in range(HPG // 2):
                        # heads A = 2*hp (partitions 0:64 of tile hp),
                        # B = 2*hp+1 (partitions 64:128); their K=64 score
                        # matmuls occupy disjoint PE row-groups and run
                        # concurrently, sharing one [128, 1024] psum tile.
                        pt = hp
                        xtA = pxt.tile([P, QW], FP32, tag="xtA")
                        xtB = pxt.tile([P, QW], FP32, tag="xtB")
                        for m in range(NT):
                            sc = psc.tile([P, 2 * QW], FP32, tag="sc")
                            pr = probs.tile([P, 2 * QW], FP32R, tag="pr")
                            for j, off in ((0, 0), (1, DH)):
                                nc.tensor.matmul(
                                    sc[:, j * QW:(j + 1) * QW],
                                    ksb[pt][off:off + DH, m * P:(m + 1) * P],
                                    qsb[pt][off:off + DH, qs],
                                    start=True, stop=True,
                                    tile_position=(off, 0))
                            nc.scalar.activation(pr[:], sc[:], AF.Exp, scale=scale)
                            for j, xt, h in ((0, xtA, 2 * hp), (1, xtB, 2 * hp + 1)):
                                nc.tensor.matmul(
                                    xt[0:DH + 1, :],
                                    vsb[m][:, h * (DH + 1):(h + 1) * (DH + 1)],
                                    pr[:, j * QW:(j + 1) * QW],
                                    start=(m == 0), stop=(m == NT - 1))
                        for xt, off in ((xtA, 0), (xtB, DH)):
                            # denominator -> SBUF, cheap approx reciprocal
                            den = small.tile([1, QW], FP32, tag="den")
                            nc.vector.tensor_copy(den[:], xt[DH:DH + 1, :])
                            rde = small.tile([1, QW], FP32, tag="rde")
                            nc.vector.reciprocal_approx_fast(out=rde[:], in_=den[:])
                            rdr = small.tile([1, QW], FP32R, tag="rdr")
                            nc.vector.tensor_copy(rdr[:], rde[:])
                            bc = pbc.tile([DH, QW], FP32, tag="bc")
                            nc.tensor.matmul(bc[:], ones[:], rdr[:],
                                             start=True, stop=True)
                            bcs = small.tile([DH, QW], FP32R, tag="bcs")
                            nc.vector.tensor_copy(bcs[:], bc[:])
                            nc.vector.tensor_mul(xq[pt][off:off + DH, :],
                                                 xt[0:DH, :], bcs[:])
                    # output projection for this q-chunk
                    for mo in range(KT):
                        po = pop.tile([P, QW], FP32, tag="po")
                        for k2 in range(CS // P):
                            nc.tensor.matmul(
                                po[:],
                                wp_sb[:, k2 * DIM + mo * P:k2 * DIM + (mo + 1) * P],
                                xq[k2][:],
                                start=(k2 == 0), stop=(k2 == CS // P - 1))
                        ost = ostp.tile([P, QW], FP32, tag="ost")
                        nc.vector.tensor_copy(ost[:], po[:])
                        nc.sync.dma_start(out=out[mo * P:(mo + 1) * P, qs],
                                          in_=ost[:])

    nc.compile()
    return nc


_CACHE = {}


def _get_program(scale: float, add_qk_bias: bool, reps: int = 1,
                 loop_reps=None):
    key = (scale, add_qk_bias, reps, loop_reps)
    if key not in _CACHE:
        _CACHE[key] = _build(scale, add_qk_bias, reps, loop_reps)
    return _CACHE[key]


def make_in_maps(query, key, value, Wq, bq, Wk, bk, Wv, bv, Wp, bp, scale):
    query = np.asarray(query, np.float32)
    key = np.asarray(key, np.float32)
    value = np.asarray(value, np.float32)
    Wq, Wk, Wv, Wp = (np.asarray(a, np.float32) for a in (Wq, Wk, Wv, Wp))
    bq, bk = np.asarray(bq, np.float32), np.asarray(bk, np.float32)
    in_maps = []
    for c in range(NCORES):
        b, g = c // HG, c % HG
        cs = slice(g * CS, (g + 1) * CS)
        bqk_arr = np.stack([bq[cs].reshape(CS // P, P),
                            bk[cs].reshape(CS // P, P)]).reshape(-1, P).T
        in_maps.append({
            "qT": np.ascontiguousarray(query[b].T),
            "kT": np.ascontiguousarray(key[b].T),
            "vT": np.ascontiguousarray(value[b].T),
            "wq": np.ascontiguousarray(Wq[cs, :].T),
            "wk": np.ascontiguousarray(Wk[cs, :].T),
            "wv": np.ascontiguousarray(Wv[cs, :].T),
            "wp": np.ascontiguousarray(Wp[:, cs].T),
            "bqk": np.ascontiguousarray(bqk_arr),
        })
    return in_maps


def combine_outputs(results, bv, bp, Wp):
    bv = np.asarray(bv, np.float32)
    bp = np.asarray(bp, np.float32)
    Wp = np.asarray(Wp, np.float32)
    out = np.empty((B, N, DIM), np.float32)
    corr = bp + bv @ Wp.T
    for b in range(B):
        acc = results[b * HG]["out"].copy()
        for g in range(1, HG):
            acc += results[b * HG + g]["out"]
        out[b] = acc.T + corr
    return out


def kernel(query, key, value, Wq, bq, Wk, bk, Wv, bv, Wp, bp, scale):
    scale_v = float(np.asarray(scale).reshape(-1)[0])
    add_qk_bias = bool(np.any(np.asarray(bq)) or np.any(np.asarray(bk)))
    nc = _get_program(scale_v, add_qk_bias)
    in_maps = make_in_maps(query, key, value, Wq, bq, Wk, bk, Wv, bv,
                           Wp, bp, scale)
    res = run_bass_kernel_spmd(nc, in_maps, list(range(NCORES))).results
    return combine_outputs(res, bv, bp, Wp)



# revision 4
# speedup vs baseline: 1.0316x; 1.0316x over previous
"""Cross-attention kernel for Trainium2, sharded over 8 NeuronCores.

Sharding: core c handles batch b = c // 4 and head-group g = c % 4
(4 of 16 heads = 256 of 1024 channels). Each core computes
  q_g = query[b] @ Wq[g].T ; k_g = key[b] @ Wk[g].T ; v_g = value[b] @ Wv[g].T
  x_g = softmax(q_g k_g^T * scale) v_g          (4 heads, independent)
  partial_g = x_g @ Wp[:, g].T                  (partial over head-group)
Host sums the 4 partials per batch and adds the bias terms (bp exactly,
bv folded through: softmax rows sum to 1 so bv contributes bv @ Wp.T).

v2 design vs the v1 baseline (which serialized QK-proj / V-proj /
attention phases because each consumed all 8 PSUM banks):
- bf16 data path end to end (fp32 PSUM accumulation). Halves DMA bytes
  and SBUF footprint; rel-err budget (2e-2) has >10x margin.
- PSUM budget lets all phases coexist: scores 2x[128,1024] (4 banks),
  PV accumulators 2x[128,512] (2 banks), shared utility ring 2x[128,512]
  (2 banks) used by every projection + the reciprocal broadcast.
- Emission order software-pipelines the whole kernel: K-proj(hp0) ->
  Q-proj(hp0,c0) -> V-proj(hp01) -> attention(c0,hp0) with remaining
  projections + output projections issued under the ACT-bound attention
  stream (exp of 4x2048x2048 scores is the ~109us/core floor).
- Tiles are split per chunk / per head-pair (qsbc, vsbA/vsbB) so Tile's
  per-tile dependency tracking doesn't fabricate false serialization.
- Scores are computed transposed (scT[m, q]) so softmax denominators
  ride as a 65th output row of the PV matmul (ones column in v).
  Score matmul pairs use tile_position row tiling (64-deep contraction,
  2 concurrent).
"""

from contextlib import ExitStack, nullcontext

import numpy as np
import ml_dtypes

import concourse.bass as bass
import concourse.mybir as mybir
import concourse.tile as tile
from concourse import bacc
from concourse.bass_utils import run_bass_kernel_spmd

B, N, DIM, H, DH = 2, 2048, 1024, 16, 64
NCORES = 8
HG = 4            # head-groups (cores per batch)
HPG = H // HG     # heads per group = 4
CS = DIM // HG    # channels per group = 256
P = 128
KT = DIM // P     # 8 contraction tiles for the projections
NT = N // P       # 16 key-token tiles
QW = 512          # q-chunk width
QC = N // QW      # 4 q-chunks
DH1 = DH + 1      # head channels + ones column

FP32 = mybir.dt.float32
BF16 = mybir.dt.bfloat16
AF = mybir.ActivationFunctionType
BF = ml_dtypes.bfloat16


def _build(scale: float, add_qk_bias: bool, reps: int = 1,
           loop_reps: int | None = None):
    nc = bacc.Bacc("TRN2", target_bir_lowering=False, debug=False,
                   num_devices=NCORES)

    qT = nc.dram_tensor("qT", [DIM, N], BF16, kind="ExternalInput").ap()
    kT = nc.dram_tensor("kT", [DIM, N], BF16, kind="ExternalInput").ap()
    vT = nc.dram_tensor("vT", [DIM, N], BF16, kind="ExternalInput").ap()
    wq = nc.dram_tensor("wq", [DIM, CS], BF16, kind="ExternalInput").ap()
    wk = nc.dram_tensor("wk", [DIM, CS], BF16, kind="ExternalInput").ap()
    wv = nc.dram_tensor("wv", [DIM, CS], BF16, kind="ExternalInput").ap()
    wp = nc.dram_tensor("wp", [CS, DIM], BF16, kind="ExternalInput").ap()
    bqk = nc.dram_tensor("bqk", [P, 2 * (CS // P)], FP32,
                         kind="ExternalInput").ap()
    out = nc.dram_tensor("out", [DIM, N], FP32, kind="ExternalOutput").ap()

    with nc.allow_low_precision(reason="bf16 matmul rounding is intended"), \
         tile.TileContext(nc) as tc, ExitStack() as stack:
        wpool = stack.enter_context(tc.tile_pool(name="wpool", bufs=1))
        persist = stack.enter_context(tc.tile_pool(name="persist", bufs=1))
        const = stack.enter_context(tc.tile_pool(name="const", bufs=1))

        # Weights resident in SBUF.
        wq_sb = wpool.tile([P, KT * CS], BF16, tag="wq")
        wk_sb = wpool.tile([P, KT * CS], BF16, tag="wk")
        wv_sb = wpool.tile([P, KT * CS], BF16, tag="wv")
        wp_sb = wpool.tile([P, (CS // P) * DIM], BF16, tag="wp")
        for k in range(KT):
            nc.sync.dma_start(out=wq_sb[:, k * CS:(k + 1) * CS],
                              in_=wq[k * P:(k + 1) * P, :])
            nc.sync.dma_start(out=wk_sb[:, k * CS:(k + 1) * CS],
                              in_=wk[k * P:(k + 1) * P, :])
            nc.sync.dma_start(out=wv_sb[:, k * CS:(k + 1) * CS],
                              in_=wv[k * P:(k + 1) * P, :])
        for k2 in range(CS // P):
            nc.sync.dma_start(out=wp_sb[:, k2 * DIM:(k2 + 1) * DIM],
                              in_=wp[k2 * P:(k2 + 1) * P, :])
        bqk_sb = const.tile([P, 2 * (CS // P)], FP32, tag="bqk")
        if add_qk_bias:
            nc.sync.dma_start(out=bqk_sb[:], in_=bqk[:])
        ones = const.tile([1, DH], BF16, tag="ones")
        nc.vector.memset(ones[:], 1.0)

        loop_cm = (tc.For_i(0, loop_reps, 1) if loop_reps else nullcontext())
        with loop_cm:
          for rep in range(reps):
            r = f"_{rep}"
            with tc.tile_pool(name="streams", bufs=1) as streams, \
                 tc.tile_pool(name="qk_sb", bufs=1) as qk_pool, \
                 tc.tile_pool(name="vsb", bufs=1) as vsb_pool, \
                 tc.tile_pool(name="probs", bufs=3) as prp, \
                 tc.tile_pool(name="xq", bufs=2) as xqp, \
                 tc.tile_pool(name="small", bufs=2) as small, \
                 tc.tile_pool(name="ost", bufs=2) as ostp, \
                 tc.tile_pool(name="psc", bufs=2, space="PSUM") as psc, \
                 tc.tile_pool(name="pxt", bufs=1, space="PSUM") as pxt, \
                 tc.tile_pool(name="putil", bufs=2, space="PSUM") as putil:

                # Stream tensors resident in SBUF (bf16).
                kt = [streams.tile([P, N], BF16, tag=f"kt{k}",
                                   name=f"kt{k}{r}") for k in range(KT)]
                vt = [streams.tile([P, N], BF16, tag=f"vt{k}",
                                   name=f"vt{k}{r}") for k in range(KT)]
                qt = [streams.tile([P, N], BF16, tag=f"qt{k}",
                                   name=f"qt{k}{r}") for k in range(KT)]
                # Projected activations.
                ksb = [qk_pool.tile([P, N], BF16, tag=f"ksb{t}",
                                    name=f"ksb{t}{r}") for t in range(2)]
                qsbc = [[qk_pool.tile([P, QW], BF16, tag=f"qsb{t}c{c}",
                                      name=f"qsb{t}c{c}{r}")
                         for c in range(QC)] for t in range(2)]
                # v token-major, 2 heads + ones column each: [tok, 2*65]
                vsbA = [vsb_pool.tile([P, 2 * DH1], BF16, tag=f"vsA{t}",
                                      name=f"vsA{t}{r}") for t in range(NT)]
                vsbB = [vsb_pool.tile([P, 2 * DH1], BF16, tag=f"vsB{t}",
                                      name=f"vsB{t}{r}") for t in range(NT)]

                for k in range(KT):
                    nc.sync.dma_start(out=kt[k][:], in_=kT[k * P:(k + 1) * P, :])
                for k in range(KT):
                    nc.sync.dma_start(out=vt[k][:], in_=vT[k * P:(k + 1) * P, :])
                for k in range(KT):
                    nc.sync.dma_start(out=qt[k][:], in_=qT[k * P:(k + 1) * P, :])

                def kq_proj_chunk(src, w_sb, hp, c, dst, bias_col, nm):
                    """Project one 512-token chunk for one head-pair."""
                    pa = putil.tile([P, QW], FP32, tag="util",
                                    name=f"pj_{nm}{hp}{c}{r}")
                    for k in range(KT):
                        nc.tensor.matmul(
                            pa[:],
                            w_sb[:, k * CS + hp * P:k * CS + (hp + 1) * P],
                            src[k][:, c * QW:(c + 1) * QW],
                            start=(k == 0), stop=(k == KT - 1))
                    if add_qk_bias:
                        nc.vector.tensor_scalar(
                            dst[:], pa[:],
                            bqk_sb[:, bias_col + hp:bias_col + hp + 1], None,
                            mybir.AluOpType.add)
                    else:
                        nc.vector.tensor_copy(dst[:], pa[:])

                def k_proj(hp):
                    for c in range(QC):
                        kq_proj_chunk(kt, wk_sb, hp, c,
                                      ksb[hp][:, c * QW:(c + 1) * QW],
                                      CS // P, "k")

                def q_proj(hp, c):
                    kq_proj_chunk(qt, wq_sb, hp, c, qsbc[hp][c][:], 0, "q")

                def v_proj(hp):
                    """Project v for one head-pair (output channels
                    hp*128:(hp+1)*128) into vsbA/vsbB token tiles."""
                    vsb = vsbA if hp == 0 else vsbB
                    for tt in range(NT):
                        pv = putil.tile([P, P], FP32, tag="util",
                                        name=f"pv{hp}{tt}{r}")
                        for k in range(KT):
                            nc.tensor.matmul(
                                pv[:],
                                vt[k][:, tt * P:(tt + 1) * P],
                                wv_sb[:, k * CS + hp * P:k * CS + (hp + 1) * P],
                                start=(k == 0), stop=(k == KT - 1))
                        dst3 = vsb[tt][:].rearrange("p (h c) -> p h c", h=2)
                        nc.vector.tensor_copy(
                            dst3[:, :, 0:DH],
                            pv[:].rearrange("p (h c) -> p h c", h=2))
                        nc.vector.memset(dst3[:, :, DH:DH1], 1.0)

                def attn(c, hp):
                    """Attention for q-chunk c, head-pair hp -> xq tile."""
                    vsb = vsbA if hp == 0 else vsbB
                    xq = xqp.tile([P, QW], BF16, tag=f"xq{hp}",
                                  name=f"xq{hp}_{c}{r}")
                    xtA = pxt.tile([P, QW], FP32, tag="xtA")
                    xtB = pxt.tile([P, QW], FP32, tag="xtB")
                    for m in range(NT):
                        sc = psc.tile([P, 2 * QW], FP32, tag="sc")
                        pr = prp.tile([P, 2 * QW], BF16, tag="pr")
                        for j, off in ((0, 0), (1, DH)):
                            nc.tensor.matmul(
                                sc[:, j * QW:(j + 1) * QW],
                                ksb[hp][off:off + DH, m * P:(m + 1) * P],
                                qsbc[hp][c][off:off + DH, :],
                                start=True, stop=True,
                                tile_position=(off, 0))
                        nc.scalar.activation(pr[:], sc[:], AF.Exp, scale=scale)
                        for j, xt in ((0, xtA), (1, xtB)):
                            nc.tensor.matmul(
                                xt[0:DH1, :],
                                vsb[m][:, j * DH1:(j + 1) * DH1],
                                pr[:, j * QW:(j + 1) * QW],
                                start=(m == 0), stop=(m == NT - 1))
                    for j, xt in ((0, xtA), (1, xtB)):
                        den = small.tile([1, QW], FP32, tag="den")
                        nc.vector.tensor_copy(den[:], xt[DH:DH1, :])
                        rde = small.tile([1, QW], FP32, tag="rde")
                        nc.vector.reciprocal_approx_fast(out=rde[:], in_=den[:])
                        rdr = small.tile([1, QW], BF16, tag="rdr")
                        nc.vector.tensor_copy(rdr[:], rde[:])
                        bc = putil.tile([DH, QW], FP32, tag="util",
                                        name=f"bc{c}{hp}{j}{r}")
                        nc.tensor.matmul(bc[:], ones[:], rdr[:],
                                         start=True, stop=True)
                        bcs = small.tile([DH, QW], FP32, tag="bcs")
                        nc.vector.tensor_copy(bcs[:], bc[:])
                        nc.vector.tensor_mul(xq[j * DH:(j + 1) * DH, :],
                                             xt[0:DH, :], bcs[:])
                    return xq

                def out_proj(c, xq0, xq1):
                    for mo in range(KT):
                        po = putil.tile([P, QW], FP32, tag="util",
                                        name=f"po{c}{mo}{r}")
                        nc.tensor.matmul(
                            po[:], wp_sb[:, mo * P:(mo + 1) * P],
                            xq0[:], start=True, stop=False)
                        nc.tensor.matmul(
                            po[:], wp_sb[:, DIM + mo * P:DIM + (mo + 1) * P],
                            xq1[:], start=False, stop=True)
                        ost = ostp.tile([P, QW], FP32, tag="ost")
                        nc.vector.tensor_copy(ost[:], po[:])
                        nc.sync.dma_start(
                            out=out[mo * P:(mo + 1) * P,
                                    c * QW:(c + 1) * QW],
                            in_=ost[:])

                # ---- software-pipelined emission schedule ----
                k_proj(0)
                q_proj(0, 0)
                v_proj(0)
                xq0 = attn(0, 0)
                k_proj(1)
                v_proj(1)
                q_proj(1, 0)
                xq1 = attn(0, 1)
                q_proj(0, 1)
                q_proj(1, 1)
                out_proj(0, xq0, xq1)
                for c in range(1, QC):
                    xq0 = attn(c, 0)
                    xq1 = attn(c, 1)
                    if c + 1 < QC:
                        q_proj(0, c + 1)
                        q_proj(1, c + 1)
                    out_proj(c, xq0, xq1)

    nc.compile()
    return nc


_CACHE = {}


def _get_program(scale: float, add_qk_bias: bool, reps: int = 1,
                 loop_reps=None):
    key = (scale, add_qk_bias, reps, loop_reps)
    if key not in _CACHE:
        _CACHE[key] = _build(scale, add_qk_bias, reps, loop_reps)
    return _CACHE[key]


def make_in_maps(query, key, value, Wq, bq, Wk, bk, Wv, bv, Wp, bp, scale):
    query = np.asarray(query, np.float32)
    key = np.asarray(key, np.float32)
    value = np.asarray(value, np.float32)
    Wq, Wk, Wv, Wp = (np.asarray(a, np.float32) for a in (Wq, Wk, Wv, Wp))
    bq, bk = np.asarray(bq, np.float32), np.asarray(bk, np.float32)
    in_maps = []
    for c in range(NCORES):
        b, g = c // HG, c % HG
        cs = slice(g * CS, (g + 1) * CS)
        bqk_arr = np.stack([bq[cs].reshape(CS // P, P),
                            bk[cs].reshape(CS // P, P)]).reshape(-1, P).T
        in_maps.append({
            "qT": np.ascontiguousarray(query[b].T).astype(BF),
            "kT": np.ascontiguousarray(key[b].T).astype(BF),
            "vT": np.ascontiguousarray(value[b].T).astype(BF),
            "wq": np.ascontiguousarray(Wq[cs, :].T).astype(BF),
            "wk": np.ascontiguousarray(Wk[cs, :].T).astype(BF),
            "wv": np.ascontiguousarray(Wv[cs, :].T).astype(BF),
            "wp": np.ascontiguousarray(Wp[:, cs].T).astype(BF),
            "bqk": np.ascontiguousarray(bqk_arr),
        })
    return in_maps


def combine_outputs(results, bv, bp, Wp):
    bv = np.asarray(bv, np.float32)
    bp = np.asarray(bp, np.float32)
    Wp = np.asarray(Wp, np.float32)
    out = np.empty((B, N, DIM), np.float32)
    corr = bp + bv @ Wp.T
    for b in range(B):
        acc = results[b * HG]["out"].copy()
        for g in range(1, HG):
            acc += results[b * HG + g]["out"]
        out[b] = acc.T + corr
    return out


def kernel(query, key, value, Wq, bq, Wk, bk, Wv, bv, Wp, bp, scale):
    scale_v = float(np.asarray(scale).reshape(-1)[0])
    add_qk_bias = bool(np.any(np.asarray(bq)) or np.any(np.asarray(bk)))
    nc = _get_program(scale_v, add_qk_bias)
    in_maps = make_in_maps(query, key, value, Wq, bq, Wk, bk, Wv, bv,
                           Wp, bp, scale)
    res = run_bass_kernel_spmd(nc, in_maps, list(range(NCORES))).results
    return combine_outputs(res, bv, bp, Wp)


# revision 8
# speedup vs baseline: 1.0898x; 1.0565x over previous
"""Cross-attention kernel for Trainium2, sharded over 8 NeuronCores.

Sharding: core c handles batch b = c // 4 and head-group g = c % 4
(4 of 16 heads = 256 of 1024 channels). Each core computes
  q_g = query[b] @ Wq[g].T ; k_g = key[b] @ Wk[g].T ; v_g = value[b] @ Wv[g].T
  x_g = softmax(q_g k_g^T * scale) v_g          (4 heads, independent)
  partial_g = x_g @ Wp[:, g].T                  (partial over head-group)
Host sums the 4 partials per batch and adds the bias terms (bp exactly,
bv folded through: softmax rows sum to 1 so bv contributes bv @ Wp.T).

v3: explicitly software-pipelined emission. The exp of the full score
matrix (4 heads x 2048 x 2048 per core) on the ACT engine is the
throughput floor (~110us/core); everything else must hide under it.
- bf16 data path (fp32 PSUM accumulation), chunk-granular DMA so the
  first attention chunk starts after ~4MB instead of ~12MB.
- Attention inner loop emits scores(m+1) BEFORE pv(m) so the PE never
  ping-pongs with ACT, and each m-slot carries a "filler" bundle
  (projection / output-projection work) sized to the PE's slack.
- PSUM budget: scores 2x[128,1024] (4 banks) + PV accumulators 2x1 bank
  + shared utility ring 2x1 bank = 8 banks, so all phases coexist.
- Per-(head-pair, chunk) K/Q tiles and per-token-tile V tiles keep
  Tile's per-tile dependency tracking exact (no false serialization).
- Scores computed transposed (scT[m,q]); softmax denominator rides as a
  65th PV output row via a ones column in v; score matmul pairs use
  tile_position row tiling (64-contraction, 2 concurrent).
"""

from contextlib import ExitStack, nullcontext

import numpy as np
import ml_dtypes

import concourse.bass as bass
import concourse.mybir as mybir
import concourse.tile as tile
from concourse import bacc
from concourse.bass_utils import run_bass_kernel_spmd

B, N, DIM, H, DH = 2, 2048, 1024, 16, 64
NCORES = 8
HG = 4            # head-groups (cores per batch)
HPG = H // HG     # heads per group = 4
CS = DIM // HG    # channels per group = 256
P = 128
KT = DIM // P     # 8 contraction tiles for the projections
NT = N // P       # 16 key-token tiles
QW = 512          # q-chunk width
QC = N // QW      # 4 q-chunks
DH1 = DH + 1      # head channels + ones column

FP32 = mybir.dt.float32
FP32R = mybir.dt.float32r
BF16 = mybir.dt.bfloat16
AF = mybir.ActivationFunctionType
BF = ml_dtypes.bfloat16


def _build(scale: float, add_qk_bias: bool, reps: int = 1,
           loop_reps: int | None = None):
    nc = bacc.Bacc("TRN2", target_bir_lowering=False, debug=False,
                   num_devices=NCORES)

    qT = nc.dram_tensor("qT", [DIM, N], BF16, kind="ExternalInput").ap()
    kT = nc.dram_tensor("kT", [DIM, N], BF16, kind="ExternalInput").ap()
    vT = nc.dram_tensor("vT", [DIM, N], BF16, kind="ExternalInput").ap()
    wq = nc.dram_tensor("wq", [DIM, CS], BF16, kind="ExternalInput").ap()
    wk = nc.dram_tensor("wk", [DIM, CS], BF16, kind="ExternalInput").ap()
    wv = nc.dram_tensor("wv", [DIM, CS], BF16, kind="ExternalInput").ap()
    wp = nc.dram_tensor("wp", [CS, DIM], BF16, kind="ExternalInput").ap()
    bqk = nc.dram_tensor("bqk", [P, 2 * (CS // P)], FP32,
                         kind="ExternalInput").ap()
    out = nc.dram_tensor("out", [DIM, N], FP32, kind="ExternalOutput").ap()

    with nc.allow_low_precision(reason="bf16 matmul rounding is intended"), \
         tile.TileContext(nc) as tc, ExitStack() as stack:
        wpool = stack.enter_context(tc.tile_pool(name="wpool", bufs=1))
        const = stack.enter_context(tc.tile_pool(name="const", bufs=1))

        # Weights resident in SBUF.
        wq_sb = wpool.tile([P, KT * CS], BF16, tag="wq")
        wk_sb = wpool.tile([P, KT * CS], BF16, tag="wk")
        wv_sb = wpool.tile([P, KT * CS], BF16, tag="wv")
        wp_sb = wpool.tile([P, (CS // P) * DIM], BF16, tag="wp")
        for k in range(KT):
            nc.sync.dma_start(out=wq_sb[:, k * CS:(k + 1) * CS],
                              in_=wq[k * P:(k + 1) * P, :])
            nc.sync.dma_start(out=wk_sb[:, k * CS:(k + 1) * CS],
                              in_=wk[k * P:(k + 1) * P, :])
            nc.sync.dma_start(out=wv_sb[:, k * CS:(k + 1) * CS],
                              in_=wv[k * P:(k + 1) * P, :])
        for k2 in range(CS // P):
            nc.sync.dma_start(out=wp_sb[:, k2 * DIM:(k2 + 1) * DIM],
                              in_=wp[k2 * P:(k2 + 1) * P, :])
        bqk_sb = const.tile([P, 2 * (CS // P)], FP32, tag="bqk")
        if add_qk_bias:
            nc.sync.dma_start(out=bqk_sb[:], in_=bqk[:])
        ones = const.tile([1, DH], BF16, tag="ones")
        nc.vector.memset(ones[:], 1.0)

        loop_cm = (tc.For_i(0, loop_reps, 1) if loop_reps else nullcontext())
        with loop_cm:
          for rep in range(reps):
            r = f"_{rep}"
            with tc.tile_pool(name="streams", bufs=1) as streams, \
                 tc.tile_pool(name="qk_sb", bufs=1) as qk_pool, \
                 tc.tile_pool(name="vsb", bufs=1) as vsb_pool, \
                 tc.tile_pool(name="probs", bufs=6) as prp, \
                 tc.tile_pool(name="xq", bufs=2) as xqp, \
                 tc.tile_pool(name="small", bufs=3) as small, \
                 tc.tile_pool(name="ost", bufs=2) as ostp, \
                 tc.tile_pool(name="psc", bufs=2, space="PSUM") as psc, \
                 tc.tile_pool(name="pxt", bufs=1, space="PSUM") as pxt, \
                 tc.tile_pool(name="putil", bufs=2, space="PSUM") as putil:

                # Stream tensors resident in SBUF (bf16), chunk-granular
                # for k/q so attention starts after ~4MB of DMA.
                ktc = [[streams.tile([P, QW], BF16, tag=f"kt{c}_{k}",
                                     name=f"kt{c}_{k}{r}")
                        for k in range(KT)] for c in range(QC)]
                qtc = [[streams.tile([P, QW], BF16, tag=f"qt{c}_{k}",
                                     name=f"qt{c}_{k}{r}")
                        for k in range(KT)] for c in range(QC)]
                vt = [streams.tile([P, N], BF16, tag=f"vt{k}",
                                   name=f"vt{k}{r}") for k in range(KT)]
                # Projected activations, per (head-pair, chunk).
                ksbc = [[qk_pool.tile([P, QW], BF16, tag=f"ksb{t}c{c}",
                                      name=f"ksb{t}c{c}{r}")
                         for c in range(QC)] for t in range(2)]
                qsbc = [[qk_pool.tile([P, QW], BF16, tag=f"qsb{t}c{c}",
                                      name=f"qsb{t}c{c}{r}")
                         for c in range(QC)] for t in range(2)]
                # v token-major, 2 heads + ones column each: [tok, 2*65]
                vsbA = [vsb_pool.tile([P, 2 * DH1], BF16, tag=f"vsA{t}",
                                      name=f"vsA{t}{r}") for t in range(NT)]
                vsbB = [vsb_pool.tile([P, 2 * DH1], BF16, tag=f"vsB{t}",
                                      name=f"vsB{t}{r}") for t in range(NT)]

                def dma_chunk(dst_c, src, c):
                    for k in range(KT):
                        nc.sync.dma_start(
                            out=dst_c[c][k][:],
                            in_=src[k * P:(k + 1) * P,
                                    c * QW:(c + 1) * QW])

                # DMA priority order: what the first attention chunk
                # needs comes first.
                dma_chunk(ktc, kT, 0)
                dma_chunk(qtc, qT, 0)
                dma_chunk(ktc, kT, 1)
                for k in range(KT):
                    nc.sync.dma_start(out=vt[k][:],
                                      in_=vT[k * P:(k + 1) * P, :])
                dma_chunk(ktc, kT, 2)
                dma_chunk(ktc, kT, 3)
                dma_chunk(qtc, qT, 1)
                dma_chunk(qtc, qT, 2)
                dma_chunk(qtc, qT, 3)

                # ---- work bundles (emitted as attention fillers) ----
                def mk_kq_proj(src_c, w_sb, hp, c, dst, bias_col, nm):
                    def emit():
                        pa = putil.tile([P, QW], FP32, tag="util",
                                        name=f"pj_{nm}{hp}{c}{r}")
                        for k in range(KT):
                            nc.tensor.matmul(
                                pa[:],
                                w_sb[:, k * CS + hp * P:k * CS + (hp + 1) * P],
                                src_c[c][k][:],
                                start=(k == 0), stop=(k == KT - 1))
                        if add_qk_bias:
                            nc.vector.tensor_scalar(
                                dst[:], pa[:],
                                bqk_sb[:, bias_col + hp:bias_col + hp + 1],
                                None, mybir.AluOpType.add)
                        else:
                            nc.vector.tensor_copy(dst[:], pa[:])
                    return emit

                def kp(hp, c):
                    return mk_kq_proj(ktc, wk_sb, hp, c, ksbc[hp][c][:],
                                      CS // P, "k")

                def qp(hp, c):
                    return mk_kq_proj(qtc, wq_sb, hp, c, qsbc[hp][c][:],
                                      0, "q")

                def vp(tt):
                    def emit():
                        pv = putil.tile([P, CS], FP32, tag="util",
                                        name=f"pv{tt}{r}")
                        for k in range(KT):
                            nc.tensor.matmul(
                                pv[:],
                                vt[k][:, tt * P:(tt + 1) * P],
                                wv_sb[:, k * CS:(k + 1) * CS],
                                start=(k == 0), stop=(k == KT - 1))
                        for hp, vsb in ((0, vsbA), (1, vsbB)):
                            dst3 = vsb[tt][:].rearrange(
                                "p (h c) -> p h c", h=2)
                            nc.vector.tensor_copy(
                                dst3[:, :, 0:DH],
                                pv[:, hp * P:(hp + 1) * P].rearrange(
                                    "p (h c) -> p h c", h=2))
                            nc.vector.memset(dst3[:, :, DH:DH1], 1.0)
                    return emit

                xqs = {}

                def op(c, mo):
                    def emit():
                        xq0, xq1 = xqs[(c, 0)], xqs[(c, 1)]
                        po = putil.tile([P, QW], FP32, tag="util",
                                        name=f"po{c}{mo}{r}")
                        nc.tensor.matmul(
                            po[:], wp_sb[:, mo * P:(mo + 1) * P],
                            xq0[:], start=True, stop=False)
                        nc.tensor.matmul(
                            po[:], wp_sb[:, DIM + mo * P:DIM + (mo + 1) * P],
                            xq1[:], start=False, stop=True)
                        ost = ostp.tile([P, QW], FP32, tag="ost")
                        nc.vector.tensor_copy(ost[:], po[:])
                        nc.sync.dma_start(
                            out=out[mo * P:(mo + 1) * P,
                                    c * QW:(c + 1) * QW],
                            in_=ost[:])
                    return emit

                def attn(c, hp, fills):
                    """Attention for q-chunk c, head-pair hp.

                    Emission per m-slot: filler bundle(s), scores(m+1),
                    exp(m), pv(m) — so PE always has scores(m+1) queued
                    while ACT runs exp(m), and fillers soak PE slack."""
                    vsb = vsbA if hp == 0 else vsbB
                    xq = xqp.tile([P, QW], BF16, tag=f"xq{hp}",
                                  name=f"xq{hp}_{c}{r}")
                    xqs[(c, hp)] = xq
                    xtA = pxt.tile([P, QW], FP32, tag="xtA")
                    xtB = pxt.tile([P, QW], FP32, tag="xtB")

                    def emit_sc(m):
                        sc = psc.tile([P, 2 * QW], FP32, tag="sc")
                        for j, off in ((0, 0), (1, DH)):
                            nc.tensor.matmul(
                                sc[:, j * QW:(j + 1) * QW],
                                ksbc[hp][m // 4][off:off + DH,
                                                 (m % 4) * P:(m % 4 + 1) * P],
                                qsbc[hp][c][off:off + DH, :],
                                start=True, stop=True,
                                tile_position=(off, 0))
                        return sc

                    scs = {0: emit_sc(0)}
                    for m in range(NT):
                        if m < len(fills):
                            for b in fills[m]:
                                b()
                        if m + 1 < NT:
                            scs[m + 1] = emit_sc(m + 1)
                        pr = prp.tile([P, 2 * QW], BF16, tag="pr")
                        nc.scalar.activation(pr[:], scs.pop(m)[:],
                                             AF.Exp, scale=scale)
                        for j, xt in ((0, xtA), (1, xtB)):
                            nc.tensor.matmul(
                                xt[0:DH1, :],
                                vsb[m][:, j * DH1:(j + 1) * DH1],
                                pr[:, j * QW:(j + 1) * QW],
                                start=(m == 0), stop=(m == NT - 1))
                    for j, xt in ((0, xtA), (1, xtB)):
                        den = small.tile([1, QW], FP32, tag="den")
                        nc.vector.tensor_copy(den[:], xt[DH:DH1, :])
                        rde = small.tile([1, QW], FP32, tag="rde")
                        nc.vector.reciprocal_approx_fast(
                            out=rde[:], in_=den[:])
                        rdr = small.tile([1, QW], BF16, tag="rdr")
                        nc.vector.tensor_copy(rdr[:], rde[:])
                        bcp = putil.tile([DH, QW], FP32, tag="util",
                                         name=f"bc{c}{hp}{j}{r}")
                        nc.tensor.matmul(bcp[:], ones[:], rdr[:],
                                         start=True, stop=True)
                        bcs = small.tile([DH, QW], FP32, tag="bcs")
                        nc.vector.tensor_copy(bcs[:], bcp[:])
                        nc.vector.tensor_mul(xq[j * DH:(j + 1) * DH, :],
                                             xt[0:DH, :], bcs[:])

                # ---- pipelined schedule ----
                kp(0, 0)()
                qp(0, 0)()
                vp(0)()
                vp(1)()
                attn(0, 0, [
                    [kp(0, 1)], [vp(2), vp(3)], [vp(4)], [vp(5)],
                    [kp(0, 2)], [vp(6)], [vp(7)], [vp(8)],
                    [kp(0, 3)], [vp(9)], [vp(10)], [vp(11)],
                    [vp(12)], [vp(13), kp(1, 0)], [vp(14)],
                    [vp(15), qp(1, 0)]])
                attn(0, 1, [
                    [kp(1, 1)], [qp(0, 1)], [qp(1, 1)], [],
                    [kp(1, 2)], [], [], [],
                    [kp(1, 3)]])
                attn(1, 0, [
                    [op(0, 0)], [op(0, 1)], [op(0, 2)], [op(0, 3)],
                    [qp(0, 2)], [], [qp(1, 2)]])
                attn(1, 1, [
                    [op(0, 4)], [op(0, 5)], [op(0, 6)], [op(0, 7)]])
                attn(2, 0, [
                    [op(1, 0)], [op(1, 1)], [op(1, 2)], [op(1, 3)],
                    [qp(0, 3)], [], [qp(1, 3)]])
                attn(2, 1, [
                    [op(1, 4)], [op(1, 5)], [op(1, 6)], [op(1, 7)]])
                attn(3, 0, [
                    [op(2, 0)], [op(2, 1)], [op(2, 2)], [op(2, 3)]])
                attn(3, 1, [
                    [op(2, 4)], [op(2, 5)], [op(2, 6)], [op(2, 7)]])
                for mo in range(KT):
                    op(3, mo)()

    nc.compile()
    return nc


_CACHE = {}


def _get_program(scale: float, add_qk_bias: bool, reps: int = 1,
                 loop_reps=None):
    key = (scale, add_qk_bias, reps, loop_reps)
    if key not in _CACHE:
        _CACHE[key] = _build(scale, add_qk_bias, reps, loop_reps)
    return _CACHE[key]


def make_in_maps(query, key, value, Wq, bq, Wk, bk, Wv, bv, Wp, bp, scale):
    query = np.asarray(query, np.float32)
    key = np.asarray(key, np.float32)
    value = np.asarray(value, np.float32)
    Wq, Wk, Wv, Wp = (np.asarray(a, np.float32) for a in (Wq, Wk, Wv, Wp))
    bq, bk = np.asarray(bq, np.float32), np.asarray(bk, np.float32)
    in_maps = []
    for c in range(NCORES):
        b, g = c // HG, c % HG
        cs = slice(g * CS, (g + 1) * CS)
        bqk_arr = np.stack([bq[cs].reshape(CS // P, P),
                            bk[cs].reshape(CS // P, P)]).reshape(-1, P).T
        in_maps.append({
            "qT": np.ascontiguousarray(query[b].T).astype(BF),
            "kT": np.ascontiguousarray(key[b].T).astype(BF),
            "vT": np.ascontiguousarray(value[b].T).astype(BF),
            "wq": np.ascontiguousarray(Wq[cs, :].T).astype(BF),
            "wk": np.ascontiguousarray(Wk[cs, :].T).astype(BF),
            "wv": np.ascontiguousarray(Wv[cs, :].T).astype(BF),
            "wp": np.ascontiguousarray(Wp[:, cs].T).astype(BF),
            "bqk": np.ascontiguousarray(bqk_arr),
        })
    return in_maps


def combine_outputs(results, bv, bp, Wp):
    bv = np.asarray(bv, np.float32)
    bp = np.asarray(bp, np.float32)
    Wp = np.asarray(Wp, np.float32)
    out = np.empty((B, N, DIM), np.float32)
    corr = bp + bv @ Wp.T
    for b in range(B):
        acc = results[b * HG]["out"].copy()
        for g in range(1, HG):
            acc += results[b * HG + g]["out"]
        out[b] = acc.T + corr
    return out


def kernel(query, key, value, Wq, bq, Wk, bk, Wv, bv, Wp, bp, scale):
    scale_v = float(np.asarray(scale).reshape(-1)[0])
    add_qk_bias = bool(np.any(np.asarray(bq)) or np.any(np.asarray(bk)))
    nc = _get_program(scale_v, add_qk_bias)
    in_maps = make_in_maps(query, key, value, Wq, bq, Wk, bk, Wv, bv,
                           Wp, bp, scale)
    res = run_bass_kernel_spmd(nc, in_maps, list(range(NCORES))).results
    return combine_outputs(res, bv, bp, Wp)
